# revision 6
# baseline (speedup 1.0000x reference)
"""Trainium2 Bass kernel: fused multi-head attention (dense transformer block).

Reference computation (per batch element b of 8, one NeuronCore each):
    qkv = x @ w_qkv.T                  # [1024, 2304]
    q, k, v = split(qkv); reshape to 12 heads x 64 dims
    s = q @ k.T (unscaled); p = softmax(s); o = p @ v
    out = concat_heads(o) @ w_fc.T + b_fc

Layout strategy (per core): transposed layout throughout — q_T/k_T are
[head_dim, seq], scores S_T[k, q] (keys on partitions), softmax denominator
via ones-column in V. Softmax skips max-subtraction (|scores| < 88).

v2 restructure (vs. 279us baseline): the kernel is paced by two engines in
dead heat — PE matmuls (~21us/head-pair at full clock) and ScalarE's exp
stream (~20.5us/pair). The baseline coupled every background matmul to the
exp drain through one shared 2-buffer PSUM tag, so PE stalled ~1.3us behind
exp constantly, dropping the PE DVFS p-state (2.4 -> 1.2 GHz) and looping.
Fixes:
  - PSUM split into 3 independent groups: scores "s" 2x[128,1024] (4 banks)
    <-> exp ping-pong; PV "pv" 2x[128,512] (2 banks); background "w"
    2x[128,512] (2 banks) drained by VectorE only.
  - P tiles (exp output) double-buffered by pair parity so exp never waits
    on the PV consumer.
  - PV split into q-half groups (h2) with mid-pair drains so it fits 2 banks.
  - Normalization without DRAM bounce and without ScalarE: DVE reciprocal
    reads the denominator row straight from PSUM, a 1-partition PE matmul
    (ones[1,64] x recip[1,512]) broadcasts it, DVE multiplies. ScalarE runs
    exp ONLY (its floor ~123us stays under the PE's ~165us).
  - Background work (qkv for pair p+1, v, w_fc prep, bias) rebalanced across
    pairs so every pair has PE work >= exp work, keeping the PE dense and
    the clock at max.
"""

import numpy as np
import concourse.bacc as bacc
import concourse.mybir as mybir
import concourse.tile as tile
from concourse.bass_utils import run_bass_kernel_spmd

SEQ = 1024
DIM = 768
H = 12
DH = 64
E = 3 * DIM  # 2304
NT = SEQ // 128  # 8  seq chunks
DT = DIM // 128  # 6  dim chunks
VA = H * (DH + 1)  # 780: v with ones column per head

f32 = mybir.dt.float32
f32r = mybir.dt.float32r
bf16 = mybir.dt.bfloat16
EXP = mybir.ActivationFunctionType.Exp


def build():
    nc = bacc.Bacc("TRN2", target_bir_lowering=False, debug=False)
    x_d = nc.dram_tensor("x", [SEQ, DIM], f32, kind="ExternalInput")
    wqkv_d = nc.dram_tensor("w_qkv", [E, DIM], f32, kind="ExternalInput")
    wfc_d = nc.dram_tensor("w_fc", [DIM, DIM], f32, kind="ExternalInput")
    bfc_d = nc.dram_tensor("b_fc", [1, DIM], f32, kind="ExternalInput")
    eye_d = nc.dram_tensor("eye", [128, 128], f32, kind="ExternalInput")
    out_d = nc.dram_tensor("out", [SEQ, DIM], f32, kind="ExternalOutput")

    with tile.TileContext(nc) as tc:
        with (
            tc.tile_pool(name="const", bufs=1) as constp,
            tc.tile_pool(name="persist", bufs=1) as persist,
            tc.tile_pool(name="work", bufs=1) as work,
            tc.tile_pool(name="ps", bufs=1, space="PSUM") as psp,
        ):
            # ---- constants ----
            eye = constp.tile([128, 128], f32, tag="eye")
            nc.sync.dma_start(eye[:], eye_d.ap())
            ones_f = constp.tile([1, 128], f32, tag="onesf")
            nc.gpsimd.memset(ones_f[:], 1.0)
            ones_r = constp.tile([1, 128], f32r, tag="onesr")
            nc.vector.tensor_copy(ones_r[:], ones_f[:])
            bias_bc = constp.tile([128, DIM], f32, tag="bbc")

            def bias_prep():
                bias_row = constp.tile([1, DIM], f32, tag="brow")
                nc.sync.dma_start(bias_row[:], bfc_d.ap())
                bias_r = constp.tile([1, DIM], f32r, tag="biasr")
                nc.vector.tensor_copy(bias_r[:], bias_row[:])
                for q in range(DT):
                    bb = psp.tile([128, 512], f32, tag="w", bufs=2, name="bb")
                    nc.tensor.matmul(bb[:, 0:128], ones_r[:],
                                     bias_r[:, q * 128:(q + 1) * 128],
                                     start=True, stop=True)
                    nc.vector.tensor_copy(bias_bc[:, q * 128:(q + 1) * 128],
                                          bb[:, 0:128])

            # persistent tensors
            va = [persist.tile([128, VA], bf16, tag=f"va{nt}", name=f"va{nt}")
                  for nt in range(NT)]
            aoT = [persist.tile([128, SEQ], bf16, tag=f"ao{j}", name=f"aoT{j}")
                   for j in range(DT)]
            wfcT = [persist.tile([128, DIM], bf16, tag=f"wfcT{j}",
                                 name=f"wfcT{j}") for j in range(DT)]
            xT = [persist.tile([128, SEQ], f32r, tag=f"xT{j}", name=f"xT{j}")
                  for j in range(DT)]
            wvT = [persist.tile([128, DIM], f32r, tag=f"wvT{j}",
                                name=f"wvT{j}") for j in range(DT)]

            # ---- x and w_v: load + transpose, groups interleaved so each
            # group's DMA loads hide under the previous group's transposes ----
            def x_group(g):
                xr4 = []
                for i in range(4):
                    nt = g * 4 + i
                    xr = work.tile([128, DIM], f32, tag=f"u{i}", bufs=1,
                                   name=f"xr{nt}")
                    nc.sync.dma_start(xr[:],
                                      x_d.ap()[nt * 128:(nt + 1) * 128, :])
                    xr4.append(xr)

                def tr():
                    for j in range(DT):
                        tag = ("w", "pv")[j % 2]
                        tp = psp.tile([128, 512], f32, tag=tag, bufs=2,
                                      name="tp")
                        for i in range(4):
                            nc.tensor.transpose(
                                tp[:, i * 128:(i + 1) * 128],
                                xr4[i][:, j * 128:(j + 1) * 128], eye[:])
                        nc.vector.tensor_copy(
                            xT[j][:, g * 512:(g + 1) * 512], tp[:])
                return tr

            def wv_group(g):
                idxs = (range(0, 4), range(4, 6))[g]
                wr4 = []
                for ii, i in enumerate(idxs):
                    wr = work.tile([128, DIM], f32, tag=f"u{2 + ii}", bufs=1,
                                   name=f"wvr{i}")
                    nc.sync.dma_start(
                        wr[:], wqkv_d.ap()[(12 + i) * 128:(13 + i) * 128, :])
                    wr4.append(wr)

                def tr():
                    for j in range(DT):
                        tag = ("w", "pv")[j % 2]
                        tp = psp.tile([128, 512], f32, tag=tag, bufs=2,
                                      name="tp")
                        for ii in range(len(wr4)):
                            nc.tensor.transpose(
                                tp[:, ii * 128:(ii + 1) * 128],
                                wr4[ii][:, j * 128:(j + 1) * 128], eye[:])
                        nc.vector.tensor_copy(
                            wvT[j][:, idxs[0] * 128:
                                   (idxs[0] + len(wr4)) * 128],
                            tp[:, 0:128 * len(wr4)])
                return tr

            tr_x0 = x_group(0)
            tr_wv0 = wv_group(0)
            tr_x0()
            tr_x1 = x_group(1)
            tr_wv0()
            tr_wv1 = wv_group(1)
            tr_x1()
            tr_wv1()

            # ---- v matmuls as self-contained parts (one (nt, h2) each) ----
            def v_part(nt, h2):
                lo, hi = (0, 512) if h2 == 0 else (512, 768)

                def go():
                    psv = psp.tile([128, 512], f32, tag="w", bufs=2,
                                   name="psv")
                    for j in range(DT):
                        nc.tensor.matmul(psv[:, 0:hi - lo],
                                         xT[j][:, nt * 128:(nt + 1) * 128],
                                         wvT[j][:, lo:hi],
                                         start=(j == 0), stop=(j == DT - 1))
                    va3 = va[nt][:].rearrange("p (h c) -> p h c", c=DH + 1)
                    if h2 == 0:
                        # ones columns for ALL heads now: PV(p) reads head
                        # 2p's ones col as early as pair 1, but h2==1 v parts
                        # land as late as pair 4.
                        nc.gpsimd.memset(va3[:, :, DH:DH + 1], 1.0)
                    nc.vector.tensor_copy(
                        va3[:, lo // DH:hi // DH, 0:DH],
                        psv[:, 0:hi - lo].rearrange("p (h c) -> p h c", c=DH))
                return go

            def wfc_parts():
                """w_fc load + PE-transpose as self-contained parts."""
                frs = {}

                def load(idxs):
                    for ii, ft in enumerate(idxs):
                        fr = work.tile([128, DIM], f32, tag=f"u{2 + ii}",
                                       bufs=1, name=f"fr{ft}")
                        nc.sync.dma_start(
                            fr[:], wfc_d.ap()[ft * 128:(ft + 1) * 128, :])
                        frs[ii] = fr

                def tgrp(idxs, js):
                    for j in js:
                        tp = psp.tile([128, 512], f32, tag="w", bufs=2,
                                      name="tp")
                        for ii in range(len(idxs)):
                            nc.tensor.transpose(
                                tp[:, ii * 128:(ii + 1) * 128],
                                frs[ii][:, j * 128:(j + 1) * 128], eye[:])
                        nc.vector.tensor_copy(
                            wfcT[j][:, idxs[0] * 128:
                                   (idxs[0] + len(idxs)) * 128],
                            tp[:, 0:128 * len(idxs)])

                g0, g1 = range(0, 4), range(4, 6)
                return [bias_prep,
                        lambda: (load(g0), tgrp(g0, range(0, 3))),
                        lambda: tgrp(g0, range(3, 6)),
                        lambda: (load(g1), tgrp(g1, range(0, 3))),
                        lambda: tgrp(g1, range(3, 6))]

            def wt_qkv_parts(p, tiles):
                """Pair p's w_qkv load/transpose + q_T/k_T matmuls as
                self-contained parts; results appear in `tiles`."""
                wq_t = [work.tile([128, 256], f32r, tag=f"wq{j}", bufs=1,
                                  name=f"wq{j}_{p}") for j in range(DT)]
                wraws = {}

                def load():
                    for ci, et in enumerate((p, 6 + p)):
                        wraw = work.tile([128, DIM], f32, tag=f"u{ci}",
                                         bufs=1, name=f"wqr{et}")
                        nc.sync.dma_start(
                            wraw[:], wqkv_d.ap()[et * 128:(et + 1) * 128, :])
                        wraws[ci] = wraw

                def tgrp(js):
                    for j in js:
                        tp = psp.tile([128, 512], f32, tag="w", bufs=2,
                                      name="tp")
                        for ci in range(2):
                            nc.tensor.transpose(
                                tp[:, ci * 128:(ci + 1) * 128],
                                wraws[ci][:, j * 128:(j + 1) * 128],
                                eye[:])
                        nc.vector.tensor_copy(wq_t[j][:], tp[:, 0:256])

                def qkmm(ci, half, h2):
                    ps = psp.tile([128, 512], f32, tag="w", bufs=2,
                                  name="psq")
                    for j in range(DT):
                        nc.tensor.matmul(
                            ps[:],
                            wq_t[j][:, ci * 128:(ci + 1) * 128],
                            xT[j][:, h2 * 512:(h2 + 1) * 512],
                            start=(j == 0), stop=(j == DT - 1))
                    if h2 == 0:
                        t = work.tile([128, SEQ], f32r,
                                      tag=f"qk_{half}{p % 2}", bufs=1,
                                      name=f"qk{half}{p}")
                        tiles[half] = t
                    nc.vector.tensor_copy(
                        tiles[half][:, h2 * 512:(h2 + 1) * 512], ps[:])

                return [lambda: (load(), tgrp(range(0, 3))),
                        lambda: tgrp(range(3, 6)),
                        lambda: qkmm(0, "q", 0), lambda: qkmm(0, "q", 1),
                        lambda: qkmm(1, "k", 0), lambda: qkmm(1, "k", 1)]

            def run_parts(parts):
                for f in parts:
                    f()

            # ---- PV + normalize machinery ----
            def pv_work(p, PT):
                """Closures for PV(p): per (h2): 16 c-x-xi matmuls into
                [65,512] po tiles (ones-row gives the softmax denominator),
                then drains. Drain = DVE recip (PSUM in) -> 1-partition PE
                broadcast matmul -> DVE multiply into aoT. No ScalarE, no
                DRAM bounce."""
                po = {}
                rtiles = {}
                bcs = {}
                items = []

                def pv_mm(xi, h2, c):
                    def go():
                        if c == 0:
                            po[(xi, h2)] = psp.tile(
                                [128, 512], f32, tag="pv", bufs=2,
                                name=f"po{xi}_{h2}")
                        hX = 2 * p + xi
                        va_h = va[c][:, hX * (DH + 1):(hX + 1) * (DH + 1)]
                        nc.tensor.matmul(
                            po[(xi, h2)][0:DH + 1, :], va_h,
                            PT[(xi, c)][:, h2 * 512:(h2 + 1) * 512],
                            start=(c == 0), stop=(c == NT - 1))
                    return go

                def drain_a(xi, h2):
                    def go():
                        r = work.tile([1, 512], f32r, tag="recip", bufs=2,
                                      name="recip")
                        # f32r out is bit-identical to f32; tagged r so the
                        # broadcast matmul's moving operand runs at full rate
                        with nc.allow_low_precision(reason="f32r==f32 bits"):
                            nc.vector.reciprocal(
                                r[:], po[(xi, h2)][DH:DH + 1, :])
                        rtiles[(xi, h2)] = r
                    return go

                def drain_b(xi, h2):
                    def go():
                        bc = psp.tile([128, 512], f32, tag="w", bufs=2,
                                      name="bc")
                        nc.tensor.matmul(bc[0:DH, :], ones_r[0:1, 0:DH],
                                         rtiles[(xi, h2)][:],
                                         start=True, stop=True)
                        st = work.tile([DH, 512], f32, tag="stg", bufs=2,
                                       name="stg")
                        nc.vector.tensor_copy(st[:], po[(xi, h2)][0:DH, :])
                        bcs[(xi, h2)] = (bc, st)
                    return go

                def drain_c(xi, h2):
                    def go():
                        bc, st = bcs[(xi, h2)]
                        nc.vector.tensor_mul(
                            aoT[p][xi * DH:(xi + 1) * DH,
                                   h2 * 512:(h2 + 1) * 512],
                            st[:], bc[0:DH, :])
                    return go

                for h2 in (0, 1):
                    for c in range(NT):
                        for xi in (0, 1):
                            items.append(pv_mm(xi, h2, c))
                    items.append(drain_a(0, h2))
                    items.append(drain_a(1, h2))
                    items.append(drain_b(0, h2))
                    items.append(drain_b(1, h2))
                    items.append(drain_c(0, h2))
                    items.append(drain_c(1, h2))
                return items

            def pair_step(p, qk, bg=()):
                """S(p) chunk-interleaved with background items; returns
                PT(p). bg items are spread over the 16 (c, xi) half-slots."""
                qt, kt = qk["q"], qk["k"]
                L = len(bg)
                PT = {}
                for c in range(NT):
                    for xi in range(2):
                        ro = xi * 64
                        ps = psp.tile([128, SEQ], f32, tag="s", bufs=2,
                                      name="ps_s")
                        for h2 in range(2):
                            nc.tensor.matmul(
                                ps[:, h2 * 512:(h2 + 1) * 512],
                                kt[ro:ro + 64, c * 128:(c + 1) * 128],
                                qt[ro:ro + 64, h2 * 512:(h2 + 1) * 512],
                                start=True, stop=True)
                        pt = persist.tile([128, SEQ], bf16,
                                          tag=f"pt{p % 2}_{xi}_{c}",
                                          name=f"pt{p}_{xi}_{c}")
                        nc.scalar.activation(pt[:], ps[:], EXP)
                        PT[(xi, c)] = pt
                        hs = 2 * c + xi
                        for i in range(L * hs // 16, L * (hs + 1) // 16):
                            bg[i]()
                return PT

            def merge(a, b):
                out, ia, ib = [], 0, 0
                while ia < len(a) or ib < len(b):
                    if ia * len(b) <= ib * len(a) and ia < len(a):
                        out.append(a[ia]); ia += 1
                    elif ib < len(b):
                        out.append(b[ib]); ib += 1
                    else:
                        out.append(a[ia]); ia += 1
                return out

            # ---- schedule ----
            qk_tiles = {p: {} for p in range(6)}
            run_parts(wt_qkv_parts(0, qk_tiles[0]))

            v_h0 = [v_part(nt, 0) for nt in range(NT)]
            v_h1 = [v_part(nt, 1) for nt in range(NT)]

            bg_sched = {
                0: merge(wt_qkv_parts(1, qk_tiles[1]), v_h0),
                1: merge(wt_qkv_parts(2, qk_tiles[2]), v_h1[0:2]),
                2: merge(wt_qkv_parts(3, qk_tiles[3]), v_h1[2:4]),
                3: merge(wt_qkv_parts(4, qk_tiles[4]), v_h1[4:6]),
                4: merge(wt_qkv_parts(5, qk_tiles[5]), v_h1[6:8]),
                5: wfc_parts(),
            }
            PT_prev = None
            for p in range(6):
                bg = list(bg_sched[p])
                if PT_prev is not None:
                    bg = merge(pv_work(p - 1, PT_prev), bg)
                PT_prev = pair_step(p, qk_tiles[p], bg)
            # tail: PV(5) + drains back-to-back
            run_parts(pv_work(5, PT_prev))

            # ---- fc + bias, natural layout ----
            for nt in range(NT):
                psy = psp.tile([128, SEQ], f32, tag="s", bufs=2, name="psy")
                for j in range(DT):
                    nc.tensor.matmul(psy[:, 0:512],
                                     aoT[j][:, nt * 128:(nt + 1) * 128],
                                     wfcT[j][:, 0:512],
                                     start=(j == 0), stop=(j == DT - 1))
                    nc.tensor.matmul(psy[:, 512:768],
                                     aoT[j][:, nt * 128:(nt + 1) * 128],
                                     wfcT[j][:, 512:768],
                                     start=(j == 0), stop=(j == DT - 1))
                y = work.tile([128, DIM], f32, tag="y_sb", bufs=2, name="y")
                nc.vector.tensor_add(y[:], psy[:, 0:DIM], bias_bc[:])
                nc.sync.dma_start(out_d.ap()[nt * 128:(nt + 1) * 128, :], y[:])

    nc.compile()
    return nc


_NC = None
LAST_RESULTS = None  # BassKernelResults of the most recent run (for profiling)


def kernel(**inputs) -> np.ndarray:
    global _NC, LAST_RESULTS
    x = np.ascontiguousarray(np.asarray(inputs["x"], dtype=np.float32))
    w_qkv = np.ascontiguousarray(np.asarray(inputs["w_qkv"], dtype=np.float32))
    w_fc = np.ascontiguousarray(np.asarray(inputs["w_fc"], dtype=np.float32))
    b_fc = np.ascontiguousarray(
        np.asarray(inputs["b_fc"], dtype=np.float32).reshape(1, DIM))
    eye = np.eye(128, dtype=np.float32)

    if _NC is None:
        _NC = build()
    nc = _NC

    in_maps = [
        {"x": np.ascontiguousarray(x[b]), "w_qkv": w_qkv, "w_fc": w_fc,
         "b_fc": b_fc, "eye": eye}
        for b in range(8)
    ]
    res = run_bass_kernel_spmd(nc, in_maps, core_ids=list(range(8)))
    LAST_RESULTS = res
    out = np.stack([r["out"] for r in res.results], axis=0)
    return out.astype(np.float32)


if __name__ == "__main__":
    rng = np.random.default_rng(0)
    ins = {
        "x": rng.standard_normal((8, SEQ, DIM), dtype=np.float32),
        "w_qkv": (rng.standard_normal((E, DIM), dtype=np.float32) * DIM ** -0.5),
        "w_fc": (rng.standard_normal((DIM, DIM), dtype=np.float32) * DIM ** -0.5),
        "b_fc": (rng.standard_normal((DIM,), dtype=np.float32) * 0.02),
    }
    out = kernel(**ins)
    print("out", out.shape, out.dtype)


# revision 14
# speedup vs baseline: 1.1878x; 1.1878x over previous
"""Trainium2 Bass kernel: fused multi-head attention (dense transformer block).

Reference computation (per batch element b of 8, one NeuronCore each):
    qkv = x @ w_qkv.T                  # [1024, 2304]
    q, k, v = split(qkv); reshape to 12 heads x 64 dims
    s = q @ k.T (unscaled); p = softmax(s); o = p @ v
    out = concat_heads(o) @ w_fc.T + b_fc

Layout strategy (per core): transposed layout throughout — q_T/k_T are
[head_dim, seq], scores S_T[k, q] (keys on partitions), softmax denominator
via ones-column in V. Softmax skips max-subtraction (|scores| < 88).

v2 restructure (vs. 279us baseline): the kernel is paced by two engines in
dead heat — PE matmuls (~21us/head-pair at full clock) and ScalarE's exp
stream (~20.5us/pair). The baseline coupled every background matmul to the
exp drain through one shared 2-buffer PSUM tag, so PE stalled ~1.3us behind
exp constantly, dropping the PE DVFS p-state (2.4 -> 1.2 GHz) and looping.
Fixes:
  - PSUM split into 3 independent groups: scores "s" 2x[128,1024] (4 banks)
    <-> exp ping-pong; PV "pv" 2x[128,512] (2 banks); background "w"
    2x[128,512] (2 banks) drained by VectorE only.
  - P tiles (exp output) double-buffered by pair parity so exp never waits
    on the PV consumer.
  - PV split into q-half groups (h2) with mid-pair drains so it fits 2 banks.
  - Normalization without DRAM bounce and without ScalarE: DVE reciprocal
    reads the denominator row straight from PSUM, a 1-partition PE matmul
    (ones[1,64] x recip[1,512]) broadcasts it, DVE multiplies. ScalarE runs
    exp ONLY (its floor ~123us stays under the PE's ~165us).
  - Background work (qkv for pair p+1, v, w_fc prep, bias) rebalanced across
    pairs so every pair has PE work >= exp work, keeping the PE dense and
    the clock at max.
"""

import numpy as np
import concourse.bacc as bacc
import concourse.mybir as mybir
import concourse.tile as tile
from concourse.bass_utils import run_bass_kernel_spmd

SEQ = 1024
DIM = 768
H = 12
DH = 64
E = 3 * DIM  # 2304
NT = SEQ // 128  # 8  seq chunks
DT = DIM // 128  # 6  dim chunks
VA = H * (DH + 1)  # 780: v with ones column per head

f32 = mybir.dt.float32
f32r = mybir.dt.float32r
bf16 = mybir.dt.bfloat16
EXP = mybir.ActivationFunctionType.Exp


def build():
    nc = bacc.Bacc("TRN2", target_bir_lowering=False, debug=False)
    x_d = nc.dram_tensor("x", [SEQ, DIM], f32, kind="ExternalInput")
    wqkv_d = nc.dram_tensor("w_qkv", [E, DIM], f32, kind="ExternalInput")
    wfc_d = nc.dram_tensor("w_fc", [DIM, DIM], f32, kind="ExternalInput")
    bfc_d = nc.dram_tensor("b_fc", [1, DIM], f32, kind="ExternalInput")
    eye_d = nc.dram_tensor("eye", [128, 128], f32, kind="ExternalInput")
    out_d = nc.dram_tensor("out", [SEQ, DIM], f32, kind="ExternalOutput")

    with tile.TileContext(nc) as tc:
        with (
            tc.tile_pool(name="const", bufs=1) as constp,
            tc.tile_pool(name="persist", bufs=1) as persist,
            tc.tile_pool(name="work", bufs=1) as work,
            tc.tile_pool(name="ps", bufs=1, space="PSUM") as psp,
        ):
            # ---- constants ----
            eye = constp.tile([128, 128], f32, tag="eye")
            nc.sync.dma_start(eye[:], eye_d.ap())
            ones_f = constp.tile([1, 128], f32, tag="onesf")
            nc.gpsimd.memset(ones_f[:], 1.0)
            ones_r = constp.tile([1, 128], f32r, tag="onesr")
            nc.vector.tensor_copy(ones_r[:], ones_f[:])
            ones_qf = constp.tile([65, DH], f32, tag="onesqf")
            nc.gpsimd.memset(ones_qf[:], 1.0)
            ones_q = constp.tile([65, DH], f32r, tag="onesq")
            nc.vector.tensor_copy(ones_q[:], ones_qf[:])
            # den stash per q-half: xi0's den lands at partition 32 (33-row
            # copy from base 0), xi1's at 64 (single-row copy). ONE wide
            # reciprocal [65,512] covers both (1-partition DVE recip
            # measured 3.3us; this shape ~0.6us). Rows 33..63 are seeded
            # once below so the wide recip never reads uninitialized
            # memory. SBUF engine APs must start at partition 0/32/64 and
            # respect 32/64-row group spans; PSUM sources are exempt.
            dstash = [constp.tile([65, 512], f32r, tag=f"dst{h}",
                                  name=f"dstash{h}")
                      for h in range(2)]
            bias_bc = constp.tile([128, DIM], f32, tag="bbc")

            def bias_prep():
                bias_row = constp.tile([1, DIM], f32, tag="brow")
                nc.sync.dma_start(bias_row[:], bfc_d.ap())
                bias_r = constp.tile([1, DIM], f32r, tag="biasr")
                nc.vector.tensor_copy(bias_r[:], bias_row[:])
                for q in range(DT):
                    bb = psp.tile([128, 512], f32, tag="w", bufs=2, name="bb")
                    nc.tensor.matmul(bb[:, 0:128], ones_r[:],
                                     bias_r[:, q * 128:(q + 1) * 128],
                                     start=True, stop=True)
                    nc.vector.tensor_copy(bias_bc[:, q * 128:(q + 1) * 128],
                                          bb[:, 0:128])

            # persistent tensors
            va = [persist.tile([128, VA], bf16, tag=f"va{nt}", name=f"va{nt}")
                  for nt in range(NT)]
            aoT = [persist.tile([128, SEQ], bf16, tag=f"ao{j}", name=f"aoT{j}")
                   for j in range(DT)]
            wfcT = [persist.tile([128, DIM], bf16, tag=f"wfcT{j}",
                                 name=f"wfcT{j}") for j in range(DT)]
            xT = [persist.tile([128, SEQ], f32r, tag=f"xT{j}", name=f"xT{j}")
                  for j in range(DT)]
            wvT = [persist.tile([128, DIM], f32r, tag=f"wvT{j}",
                                name=f"wvT{j}") for j in range(DT)]

            # ---- x and w_v: load + transpose, groups interleaved so each
            # group's DMA loads hide under the previous group's transposes ----
            def x_group(g):
                xr4 = []
                for i in range(4):
                    nt = g * 4 + i
                    xr = work.tile([128, DIM], f32, tag=f"u{i}", bufs=1,
                                   name=f"xr{nt}")
                    nc.sync.dma_start(xr[:],
                                      x_d.ap()[nt * 128:(nt + 1) * 128, :])
                    xr4.append(xr)

                def tr():
                    for j in range(DT):
                        tag = ("w", "pv")[j % 2]
                        tp = psp.tile([128, 512], f32, tag=tag, bufs=2,
                                      name="tp")
                        for i in range(4):
                            nc.tensor.transpose(
                                tp[:, i * 128:(i + 1) * 128],
                                xr4[i][:, j * 128:(j + 1) * 128], eye[:])
                        nc.vector.tensor_copy(
                            xT[j][:, g * 512:(g + 1) * 512], tp[:])
                return tr

            def wv_group(g):
                idxs = (range(0, 4), range(4, 6))[g]
                wr4 = []
                for ii, i in enumerate(idxs):
                    wr = work.tile([128, DIM], f32, tag=f"u{2 + ii}", bufs=1,
                                   name=f"wvr{i}")
                    nc.sync.dma_start(
                        wr[:], wqkv_d.ap()[(12 + i) * 128:(13 + i) * 128, :])
                    wr4.append(wr)

                def tr():
                    for j in range(DT):
                        tag = ("w", "pv")[j % 2]
                        tp = psp.tile([128, 512], f32, tag=tag, bufs=2,
                                      name="tp")
                        for ii in range(len(wr4)):
                            nc.tensor.transpose(
                                tp[:, ii * 128:(ii + 1) * 128],
                                wr4[ii][:, j * 128:(j + 1) * 128], eye[:])
                        nc.vector.tensor_copy(
                            wvT[j][:, idxs[0] * 128:
                                   (idxs[0] + len(wr4)) * 128],
                            tp[:, 0:128 * len(wr4)])
                return tr

            def seed_dstash():
                for h in range(2):
                    nc.vector.tensor_copy(dstash[h][0:DH, :],
                                          xT[0][0:DH, 0:512])

            tr_x0 = x_group(0)
            tr_wv0 = wv_group(0)
            tr_x0()
            tr_x1 = x_group(1)
            tr_wv0()
            tr_wv1 = wv_group(1)
            tr_x1()
            tr_wv1()
            seed_dstash()

            # ---- v matmuls as self-contained parts (one (nt, h2) each) ----
            def v_part(nt, h2):
                lo, hi = (0, 512) if h2 == 0 else (512, 768)

                def go():
                    psv = psp.tile([128, 512], f32, tag="w", bufs=2,
                                   name="psv")
                    for j in range(DT):
                        nc.tensor.matmul(psv[:, 0:hi - lo],
                                         xT[j][:, nt * 128:(nt + 1) * 128],
                                         wvT[j][:, lo:hi],
                                         start=(j == 0), stop=(j == DT - 1))
                    va3 = va[nt][:].rearrange("p (h c) -> p h c", c=DH + 1)
                    if h2 == 0:
                        # ones columns for ALL heads now: PV(p) reads head
                        # 2p's ones col as early as pair 1, but h2==1 v parts
                        # land as late as pair 4.
                        nc.gpsimd.memset(va3[:, :, DH:DH + 1], 1.0)
                    nc.vector.tensor_copy(
                        va3[:, lo // DH:hi // DH, 0:DH],
                        psv[:, 0:hi - lo].rearrange("p (h c) -> p h c", c=DH))
                return go

            def wfc_parts():
                """w_fc load + PE-transpose as self-contained parts."""
                frs = {}

                def load(idxs):
                    for ii, ft in enumerate(idxs):
                        fr = work.tile([128, DIM], f32, tag=f"u{2 + ii}",
                                       bufs=1, name=f"fr{ft}")
                        nc.sync.dma_start(
                            fr[:], wfc_d.ap()[ft * 128:(ft + 1) * 128, :])
                        frs[ii] = fr

                def tgrp(idxs, js):
                    for j in js:
                        tp = psp.tile([128, 512], f32, tag="w", bufs=2,
                                      name="tp")
                        for ii in range(len(idxs)):
                            nc.tensor.transpose(
                                tp[:, ii * 128:(ii + 1) * 128],
                                frs[ii][:, j * 128:(j + 1) * 128], eye[:])
                        nc.vector.tensor_copy(
                            wfcT[j][:, idxs[0] * 128:
                                   (idxs[0] + len(idxs)) * 128],
                            tp[:, 0:128 * len(idxs)])

                g0, g1 = range(0, 4), range(4, 6)
                return [bias_prep,
                        lambda: (load(g0), tgrp(g0, range(0, 3))),
                        lambda: tgrp(g0, range(3, 6)),
                        lambda: (load(g1), tgrp(g1, range(0, 3))),
                        lambda: tgrp(g1, range(3, 6))]

            def wt_qkv_parts(p, tiles):
                """Pair p's w_qkv load/transpose + q_T/k_T matmuls as
                self-contained parts; results appear in `tiles`."""
                wq_t = [work.tile([128, 256], f32r, tag=f"wq{j}", bufs=1,
                                  name=f"wq{j}_{p}") for j in range(DT)]
                wraws = {}

                def load():
                    for ci, et in enumerate((p, 6 + p)):
                        wraw = work.tile([128, DIM], f32, tag=f"u{ci}",
                                         bufs=1, name=f"wqr{et}")
                        nc.sync.dma_start(
                            wraw[:], wqkv_d.ap()[et * 128:(et + 1) * 128, :])
                        wraws[ci] = wraw

                def tgrp(js):
                    for j in js:
                        tp = psp.tile([128, 512], f32, tag="w", bufs=2,
                                      name="tp")
                        for ci in range(2):
                            nc.tensor.transpose(
                                tp[:, ci * 128:(ci + 1) * 128],
                                wraws[ci][:, j * 128:(j + 1) * 128],
                                eye[:])
                        nc.vector.tensor_copy(wq_t[j][:], tp[:, 0:256])

                def qkmm(ci, half, h2):
                    ps = psp.tile([128, 512], f32, tag="w", bufs=2,
                                  name="psq")
                    for j in range(DT):
                        nc.tensor.matmul(
                            ps[:],
                            wq_t[j][:, ci * 128:(ci + 1) * 128],
                            xT[j][:, h2 * 512:(h2 + 1) * 512],
                            start=(j == 0), stop=(j == DT - 1))
                    if h2 == 0:
                        t = work.tile([128, SEQ], f32r,
                                      tag=f"qk_{half}{p % 2}", bufs=1,
                                      name=f"qk{half}{p}")
                        tiles[half] = t
                    nc.vector.tensor_copy(
                        tiles[half][:, h2 * 512:(h2 + 1) * 512], ps[:])

                return [lambda: (load(), tgrp(range(0, 3))),
                        lambda: tgrp(range(3, 6)),
                        lambda: qkmm(0, "q", 0), lambda: qkmm(0, "q", 1),
                        lambda: qkmm(1, "k", 0), lambda: qkmm(1, "k", 1)]

            def run_parts(parts):
                for f in parts:
                    f()

            # ---- PV + normalize machinery ----
            def pv_work(p, PT):
                """Closures for PV(p): per (h2): 16 c-x-xi matmuls into
                [65,512] po tiles (ones-row gives the softmax denominator),
                then drains. Drain = DVE recip (PSUM in) -> 1-partition PE
                broadcast matmul -> DVE multiply into aoT. No ScalarE, no
                DRAM bounce."""
                po = {}
                bcs = {}
                items = []

                def pv_mm(xi, h2, c):
                    def go():
                        if c == 0:
                            po[(xi, h2)] = psp.tile(
                                [128, 512], f32, tag="pv", bufs=2,
                                name=f"po{xi}_{h2}")
                        hX = 2 * p + xi
                        va_h = va[c][:, hX * (DH + 1):(hX + 1) * (DH + 1)]
                        nc.tensor.matmul(
                            po[(xi, h2)][0:DH + 1, :], va_h,
                            PT[(xi, c)][:, h2 * 512:(h2 + 1) * 512],
                            start=(c == 0), stop=(c == NT - 1))
                    return go

                def drain_a(xi, h2):
                    def go():
                        b0 = 32 + 32 * xi
                        nc.vector.tensor_copy(dstash[h2][b0:b0 + 1, :],
                                              po[(xi, h2)][DH:DH + 1, :])
                    return go

                def drain_r(h2):
                    def go():
                        # one WIDE reciprocal for both xi (1-partition DVE
                        # reciprocal measured 3.3us; this shape ~0.6us)
                        with nc.allow_low_precision(reason="f32r==f32 bits"):
                            nc.vector.reciprocal(dstash[h2][:], dstash[h2][:])
                    return go

                def drain_b(xi, h2):
                    def go():
                        b0 = 32 + 32 * xi
                        bc = psp.tile([128, 512], f32, tag="w", bufs=2,
                                      name="bc")
                        nc.tensor.matmul(bc[0:DH, :],
                                         ones_q[b0:b0 + 1, :],
                                         dstash[h2][b0:b0 + 1, :],
                                         start=True, stop=True)
                        st = work.tile([DH, 512], f32, tag="stg", bufs=2,
                                       name="stg")
                        nc.vector.tensor_copy(st[:], po[(xi, h2)][0:DH, :])
                        bcs[(xi, h2)] = (bc, st)
                    return go

                def drain_c(xi, h2):
                    def go():
                        bc, st = bcs[(xi, h2)]
                        nc.vector.tensor_mul(
                            aoT[p][xi * DH:(xi + 1) * DH,
                                   h2 * 512:(h2 + 1) * 512],
                            st[:], bc[0:DH, :])
                    return go

                for h2 in (0, 1):
                    for c in range(NT):
                        for xi in (0, 1):
                            items.append(pv_mm(xi, h2, c))
                    items.append(drain_a(0, h2))
                    items.append(drain_a(1, h2))
                    items.append(drain_r(h2))
                    items.append(drain_b(0, h2))
                    items.append(drain_b(1, h2))
                    items.append(drain_c(0, h2))
                    items.append(drain_c(1, h2))
                return items

            def pair_step(p, qk, bg=()):
                """S(p) chunk-interleaved with background items; returns
                PT(p). bg items are spread over the 16 (c, xi) half-slots."""
                qt, kt = qk["q"], qk["k"]
                L = len(bg)
                PT = {}
                for c in range(NT):
                    for xi in range(2):
                        ro = xi * 64
                        ps = psp.tile([128, SEQ], f32, tag="s", bufs=2,
                                      name="ps_s")
                        for h2 in range(2):
                            nc.tensor.matmul(
                                ps[:, h2 * 512:(h2 + 1) * 512],
                                kt[ro:ro + 64, c * 128:(c + 1) * 128],
                                qt[ro:ro + 64, h2 * 512:(h2 + 1) * 512],
                                start=True, stop=True)
                        pt = persist.tile([128, SEQ], bf16,
                                          tag=f"pt{p % 2}_{xi}_{c}",
                                          name=f"pt{p}_{xi}_{c}")
                        nc.scalar.activation(pt[:], ps[:], EXP)
                        PT[(xi, c)] = pt
                        hs = 2 * c + xi
                        for i in range(L * hs // 16, L * (hs + 1) // 16):
                            bg[i]()
                return PT

            def merge(a, b):
                out, ia, ib = [], 0, 0
                while ia < len(a) or ib < len(b):
                    if ia * len(b) <= ib * len(a) and ia < len(a):
                        out.append(a[ia]); ia += 1
                    elif ib < len(b):
                        out.append(b[ib]); ib += 1
                    else:
                        out.append(a[ia]); ia += 1
                return out

            # ---- schedule ----
            qk_tiles = {p: {} for p in range(6)}
            run_parts(wt_qkv_parts(0, qk_tiles[0]))

            v_h0 = [v_part(nt, 0) for nt in range(NT)]
            v_h1 = [v_part(nt, 1) for nt in range(NT)]

            bg_sched = {
                0: merge(wt_qkv_parts(1, qk_tiles[1]), v_h0),
                1: merge(wt_qkv_parts(2, qk_tiles[2]), v_h1[0:2]),
                2: merge(wt_qkv_parts(3, qk_tiles[3]), v_h1[2:4]),
                3: merge(wt_qkv_parts(4, qk_tiles[4]), v_h1[4:6]),
                4: merge(wt_qkv_parts(5, qk_tiles[5]), v_h1[6:8]),
                5: wfc_parts(),
            }
            PT_prev = None
            for p in range(6):
                bg = list(bg_sched[p])
                if PT_prev is not None:
                    bg = merge(pv_work(p - 1, PT_prev), bg)
                PT_prev = pair_step(p, qk_tiles[p], bg)
            # tail: PV(5) + drains back-to-back
            run_parts(pv_work(5, PT_prev))

            # ---- fc + bias, natural layout ----
            for nt in range(NT):
                psy = psp.tile([128, SEQ], f32, tag="s", bufs=2, name="psy")
                for j in range(DT):
                    nc.tensor.matmul(psy[:, 0:512],
                                     aoT[j][:, nt * 128:(nt + 1) * 128],
                                     wfcT[j][:, 0:512],
                                     start=(j == 0), stop=(j == DT - 1))
                    nc.tensor.matmul(psy[:, 512:768],
                                     aoT[j][:, nt * 128:(nt + 1) * 128],
                                     wfcT[j][:, 512:768],
                                     start=(j == 0), stop=(j == DT - 1))
                y = work.tile([128, DIM], f32, tag="y_sb", bufs=2, name="y")
                nc.vector.tensor_add(y[:], psy[:, 0:DIM], bias_bc[:])
                nc.sync.dma_start(out_d.ap()[nt * 128:(nt + 1) * 128, :], y[:])

    nc.compile()
    return nc


_NC = None
LAST_RESULTS = None  # BassKernelResults of the most recent run (for profiling)


def kernel(**inputs) -> np.ndarray:
    global _NC, LAST_RESULTS
    x = np.ascontiguousarray(np.asarray(inputs["x"], dtype=np.float32))
    w_qkv = np.ascontiguousarray(np.asarray(inputs["w_qkv"], dtype=np.float32))
    w_fc = np.ascontiguousarray(np.asarray(inputs["w_fc"], dtype=np.float32))
    b_fc = np.ascontiguousarray(
        np.asarray(inputs["b_fc"], dtype=np.float32).reshape(1, DIM))
    eye = np.eye(128, dtype=np.float32)

    if _NC is None:
        _NC = build()
    nc = _NC

    in_maps = [
        {"x": np.ascontiguousarray(x[b]), "w_qkv": w_qkv, "w_fc": w_fc,
         "b_fc": b_fc, "eye": eye}
        for b in range(8)
    ]
    res = run_bass_kernel_spmd(nc, in_maps, core_ids=list(range(8)))
    LAST_RESULTS = res
    out = np.stack([r["out"] for r in res.results], axis=0)
    return out.astype(np.float32)


if __name__ == "__main__":
    rng = np.random.default_rng(0)
    ins = {
        "x": rng.standard_normal((8, SEQ, DIM), dtype=np.float32),
        "w_qkv": (rng.standard_normal((E, DIM), dtype=np.float32) * DIM ** -0.5),
        "w_fc": (rng.standard_normal((DIM, DIM), dtype=np.float32) * DIM ** -0.5),
        "b_fc": (rng.standard_normal((DIM,), dtype=np.float32) * 0.02),
    }
    out = kernel(**ins)
    print("out", out.shape, out.dtype)


# revision 18
# speedup vs baseline: 1.2153x; 1.0232x over previous
"""Trainium2 Bass kernel: fused multi-head attention (dense transformer block).

Reference computation (per batch element b of 8, one NeuronCore each):
    qkv = x @ w_qkv.T                  # [1024, 2304]
    q, k, v = split(qkv); reshape to 12 heads x 64 dims
    s = q @ k.T (unscaled); p = softmax(s); o = p @ v
    out = concat_heads(o) @ w_fc.T + b_fc

Layout strategy (per core): transposed layout throughout — q_T/k_T are
[head_dim, seq], scores S_T[k, q] (keys on partitions), softmax denominator
via ones-column in V. Softmax skips max-subtraction (|scores| < 88).

v2 restructure (vs. 279us baseline): the kernel is paced by two engines in
dead heat — PE matmuls (~21us/head-pair at full clock) and ScalarE's exp
stream (~20.5us/pair). The baseline coupled every background matmul to the
exp drain through one shared 2-buffer PSUM tag, so PE stalled ~1.3us behind
exp constantly, dropping the PE DVFS p-state (2.4 -> 1.2 GHz) and looping.
Fixes:
  - PSUM split into 3 independent groups: scores "s" 2x[128,1024] (4 banks)
    <-> exp ping-pong; PV "pv" 2x[128,512] (2 banks); background "w"
    2x[128,512] (2 banks) drained by VectorE only.
  - P tiles (exp output) double-buffered by pair parity so exp never waits
    on the PV consumer.
  - PV split into q-half groups (h2) with mid-pair drains so it fits 2 banks.
  - Normalization without DRAM bounce and without ScalarE: DVE reciprocal
    reads the denominator row straight from PSUM, a 1-partition PE matmul
    (ones[1,64] x recip[1,512]) broadcasts it, DVE multiplies. ScalarE runs
    exp ONLY (its floor ~123us stays under the PE's ~165us).
  - Background work (qkv for pair p+1, v, w_fc prep, bias) rebalanced across
    pairs so every pair has PE work >= exp work, keeping the PE dense and
    the clock at max.
"""

import numpy as np
import concourse.bacc as bacc
import concourse.mybir as mybir
import concourse.tile as tile
from concourse.bass_utils import run_bass_kernel_spmd

SEQ = 1024
DIM = 768
H = 12
DH = 64
E = 3 * DIM  # 2304
NT = SEQ // 128  # 8  seq chunks
DT = DIM // 128  # 6  dim chunks
VA = H * (DH + 1)  # 780: v with ones column per head

f32 = mybir.dt.float32
f32r = mybir.dt.float32r
bf16 = mybir.dt.bfloat16
EXP = mybir.ActivationFunctionType.Exp


def build():
    nc = bacc.Bacc("TRN2", target_bir_lowering=False, debug=False)
    x_d = nc.dram_tensor("x", [SEQ, DIM], f32, kind="ExternalInput")
    wqkv_d = nc.dram_tensor("w_qkv", [E, DIM], f32, kind="ExternalInput")
    wfc_d = nc.dram_tensor("w_fc", [DIM, DIM], f32, kind="ExternalInput")
    bfc_d = nc.dram_tensor("b_fc", [1, DIM], f32, kind="ExternalInput")
    eye_d = nc.dram_tensor("eye", [128, 128], f32, kind="ExternalInput")
    out_d = nc.dram_tensor("out", [SEQ, DIM], f32, kind="ExternalOutput")

    with tile.TileContext(nc) as tc:
        with (
            tc.tile_pool(name="const", bufs=1) as constp,
            tc.tile_pool(name="persist", bufs=1) as persist,
            tc.tile_pool(name="work", bufs=1) as work,
            tc.tile_pool(name="ps", bufs=1, space="PSUM") as psp,
        ):
            # ---- constants ----
            eye = constp.tile([128, 128], f32, tag="eye")
            nc.sync.dma_start(eye[:], eye_d.ap())
            ones_f = constp.tile([1, 128], f32, tag="onesf")
            nc.gpsimd.memset(ones_f[:], 1.0)
            ones_r = constp.tile([1, 128], f32r, tag="onesr")
            nc.vector.tensor_copy(ones_r[:], ones_f[:])
            ones_qf = constp.tile([65, DH], f32, tag="onesqf")
            nc.gpsimd.memset(ones_qf[:], 1.0)
            ones_q = constp.tile([65, DH], f32r, tag="onesq")
            nc.vector.tensor_copy(ones_q[:], ones_qf[:])
            # den stash per q-half: xi0's den lands at partition 32 (33-row
            # copy from base 0), xi1's at 64 (single-row copy). ONE wide
            # reciprocal [65,512] covers both (1-partition DVE recip
            # measured 3.3us; this shape ~0.6us). Rows 33..63 are seeded
            # once below so the wide recip never reads uninitialized
            # memory. SBUF engine APs must start at partition 0/32/64 and
            # respect 32/64-row group spans; PSUM sources are exempt.
            dstash = [constp.tile([65, 512], f32, tag=f"dst{h}",
                                  name=f"dstash{h}")
                      for h in range(2)]
            bias_bc = constp.tile([128, DIM], f32, tag="bbc")

            def bias_prep():
                bias_row = constp.tile([1, DIM], f32, tag="brow")
                nc.sync.dma_start(bias_row[:], bfc_d.ap())
                bias_r = constp.tile([1, DIM], f32r, tag="biasr")
                nc.vector.tensor_copy(bias_r[:], bias_row[:])
                for q in range(DT):
                    bb = psp.tile([128, 512], f32, tag="w", bufs=2, name="bb")
                    nc.tensor.matmul(bb[:, 0:128], ones_r[:],
                                     bias_r[:, q * 128:(q + 1) * 128],
                                     start=True, stop=True)
                    nc.vector.tensor_copy(bias_bc[:, q * 128:(q + 1) * 128],
                                          bb[:, 0:128])

            # persistent tensors
            va = [persist.tile([128, VA], bf16, tag=f"va{nt}", name=f"va{nt}")
                  for nt in range(NT)]
            aoT = [persist.tile([128, SEQ], bf16, tag=f"ao{j}", name=f"aoT{j}")
                   for j in range(DT)]
            wfcT = [persist.tile([128, DIM], bf16, tag=f"wfcT{j}",
                                 name=f"wfcT{j}") for j in range(DT)]
            xT = [persist.tile([128, SEQ], f32r, tag=f"xT{j}", name=f"xT{j}")
                  for j in range(DT)]
            wvT = [persist.tile([128, DIM], f32r, tag=f"wvT{j}",
                                name=f"wvT{j}") for j in range(DT)]

            # ---- x and w_v: load + transpose, groups interleaved so each
            # group's DMA loads hide under the previous group's transposes ----
            def x_group(g):
                xr4 = []
                for i in range(4):
                    nt = g * 4 + i
                    xr = work.tile([128, DIM], f32, tag=f"u{i}", bufs=1,
                                   name=f"xr{nt}")
                    nc.sync.dma_start(xr[:],
                                      x_d.ap()[nt * 128:(nt + 1) * 128, :])
                    xr4.append(xr)

                def tr():
                    for j in range(DT):
                        tag = ("w", "pv")[j % 2]
                        tp = psp.tile([128, 512], f32, tag=tag, bufs=2,
                                      name="tp")
                        for i in range(4):
                            nc.tensor.transpose(
                                tp[:, i * 128:(i + 1) * 128],
                                xr4[i][:, j * 128:(j + 1) * 128], eye[:])
                        nc.vector.tensor_copy(
                            xT[j][:, g * 512:(g + 1) * 512], tp[:])
                return tr

            def wv_group(g):
                # pairs of w_v rows staged in the fc-tail's y_sb buffers
                # (disjoint lifetimes) so these DMAs run parallel to the x
                # loads instead of queueing behind the x staging tags
                wr2 = []
                for i in range(2 * g, 2 * g + 2):
                    wr = work.tile([128, DIM], f32, tag="y_sb", bufs=2,
                                   name=f"wvr{i}")
                    nc.sync.dma_start(
                        wr[:], wqkv_d.ap()[(12 + i) * 128:(13 + i) * 128, :])
                    wr2.append(wr)

                def tr():
                    for j in range(DT):
                        tag = ("w", "pv")[j % 2]
                        tp = psp.tile([128, 512], f32, tag=tag, bufs=2,
                                      name="tp")
                        for ii in range(2):
                            nc.tensor.transpose(
                                tp[:, ii * 128:(ii + 1) * 128],
                                wr2[ii][:, j * 128:(j + 1) * 128], eye[:])
                        nc.vector.tensor_copy(
                            wvT[j][:, 2 * g * 128:(2 * g + 2) * 128],
                            tp[:, 0:256])
                return tr

            def seed_dstash():
                for h in range(2):
                    nc.vector.tensor_copy(dstash[h][0:DH, :],
                                          xT[0][0:DH, 0:512])

            tr_x0 = x_group(0)
            tr_wvA = wv_group(0)
            tr_x0()
            tr_x1 = x_group(1)
            tr_wvA()
            tr_wvB = wv_group(1)
            tr_x1()
            tr_wvC = wv_group(2)
            tr_wvB()
            tr_wvC()
            seed_dstash()

            # ---- v matmuls as self-contained parts (one (nt, h2) each) ----
            def v_part(nt, h2):
                lo, hi = (0, 512) if h2 == 0 else (512, 768)

                def go():
                    psv = psp.tile([128, 512], f32, tag="w", bufs=2,
                                   name="psv")
                    for j in range(DT):
                        nc.tensor.matmul(psv[:, 0:hi - lo],
                                         xT[j][:, nt * 128:(nt + 1) * 128],
                                         wvT[j][:, lo:hi],
                                         start=(j == 0), stop=(j == DT - 1))
                    va3 = va[nt][:].rearrange("p (h c) -> p h c", c=DH + 1)
                    if h2 == 0:
                        # ones columns for ALL heads now: PV(p) reads head
                        # 2p's ones col as early as pair 1, but h2==1 v parts
                        # land as late as pair 4.
                        nc.gpsimd.memset(va3[:, :, DH:DH + 1], 1.0)
                    nc.vector.tensor_copy(
                        va3[:, lo // DH:hi // DH, 0:DH],
                        psv[:, 0:hi - lo].rearrange("p (h c) -> p h c", c=DH))
                return go

            def wfc_parts():
                """w_fc load + PE-transpose as self-contained parts."""
                frs = {}

                def load(idxs):
                    for ii, ft in enumerate(idxs):
                        fr = work.tile([128, DIM], f32, tag=f"u{2 + ii}",
                                       bufs=1, name=f"fr{ft}")
                        nc.sync.dma_start(
                            fr[:], wfc_d.ap()[ft * 128:(ft + 1) * 128, :])
                        frs[ii] = fr

                def tgrp(idxs, js):
                    for j in js:
                        tp = psp.tile([128, 512], f32, tag="w", bufs=2,
                                      name="tp")
                        for ii in range(len(idxs)):
                            nc.tensor.transpose(
                                tp[:, ii * 128:(ii + 1) * 128],
                                frs[ii][:, j * 128:(j + 1) * 128], eye[:])
                        nc.vector.tensor_copy(
                            wfcT[j][:, idxs[0] * 128:
                                   (idxs[0] + len(idxs)) * 128],
                            tp[:, 0:128 * len(idxs)])

                g0, g1 = range(0, 4), range(4, 6)
                return [bias_prep,
                        lambda: (load(g0), tgrp(g0, range(0, 3))),
                        lambda: tgrp(g0, range(3, 6)),
                        lambda: (load(g1), tgrp(g1, range(0, 3))),
                        lambda: tgrp(g1, range(3, 6))]

            def wt_qkv_parts(p, tiles):
                """Pair p's w_qkv load/transpose + q_T/k_T matmuls as
                self-contained parts; results appear in `tiles`."""
                wq_t = [work.tile([128, 256], f32r, tag=f"wq{j}", bufs=1,
                                  name=f"wq{j}_{p}") for j in range(DT)]
                wraws = {}

                def load():
                    for ci, et in enumerate((p, 6 + p)):
                        wraw = work.tile([128, DIM], f32, tag=f"u{ci}",
                                         bufs=1, name=f"wqr{et}")
                        nc.sync.dma_start(
                            wraw[:], wqkv_d.ap()[et * 128:(et + 1) * 128, :])
                        wraws[ci] = wraw

                def tgrp(js):
                    for j in js:
                        tp = psp.tile([128, 512], f32, tag="w", bufs=2,
                                      name="tp")
                        for ci in range(2):
                            nc.tensor.transpose(
                                tp[:, ci * 128:(ci + 1) * 128],
                                wraws[ci][:, j * 128:(j + 1) * 128],
                                eye[:])
                        nc.vector.tensor_copy(wq_t[j][:], tp[:, 0:256])

                def qkmm(ci, half, h2):
                    ps = psp.tile([128, 512], f32, tag="w", bufs=2,
                                  name="psq")
                    for j in range(DT):
                        nc.tensor.matmul(
                            ps[:],
                            wq_t[j][:, ci * 128:(ci + 1) * 128],
                            xT[j][:, h2 * 512:(h2 + 1) * 512],
                            start=(j == 0), stop=(j == DT - 1))
                    if h2 == 0:
                        t = work.tile([128, SEQ], f32r,
                                      tag=f"qk_{half}{p % 2}", bufs=1,
                                      name=f"qk{half}{p}")
                        tiles[half] = t
                    nc.vector.tensor_copy(
                        tiles[half][:, h2 * 512:(h2 + 1) * 512], ps[:])

                return [lambda: (load(), tgrp(range(0, 3))),
                        lambda: tgrp(range(3, 6)),
                        lambda: qkmm(0, "q", 0), lambda: qkmm(0, "q", 1),
                        lambda: qkmm(1, "k", 0), lambda: qkmm(1, "k", 1)]

            def run_parts(parts):
                for f in parts:
                    f()

            # ---- PV + normalize machinery ----
            def pv_work(p, PT):
                """Closures for PV(p): per (h2): 16 c-x-xi matmuls into
                [65,512] po tiles (ones-row gives the softmax denominator),
                then drains. Drain = DVE recip (PSUM in) -> 1-partition PE
                broadcast matmul -> DVE multiply into aoT. No ScalarE, no
                DRAM bounce."""
                po = {}
                bcs = {}
                dsrr = {}
                items = []

                def pv_mm(xi, h2, c):
                    def go():
                        if c == 0:
                            po[(xi, h2)] = psp.tile(
                                [128, 512], f32, tag="pv", bufs=2,
                                name=f"po{xi}_{h2}")
                        hX = 2 * p + xi
                        va_h = va[c][:, hX * (DH + 1):(hX + 1) * (DH + 1)]
                        nc.tensor.matmul(
                            po[(xi, h2)][0:DH + 1, :], va_h,
                            PT[(xi, c)][:, h2 * 512:(h2 + 1) * 512],
                            start=(c == 0), stop=(c == NT - 1))
                    return go

                def drain_a(xi, h2):
                    def go():
                        b0 = 32 + 32 * xi
                        nc.vector.tensor_copy(dstash[h2][b0:b0 + 1, :],
                                              po[(xi, h2)][DH:DH + 1, :])
                    return go

                def drain_r(h2):
                    def go():
                        # DVE reciprocal costs ~6.4ns/free-element regardless
                        # of partitions (measured 3.3us for 512-wide);
                        # approx_fast is a single custom-DVE op (~18 good
                        # bits, den is in [1,1e33] so edge cases are safe)
                        nc.vector.reciprocal_approx_fast(
                            out=dstash[h2][:], in_=dstash[h2][:])
                        # rounded f32r copy: the f32r broadcast matmul needs
                        # an f32r-producing instruction (BIR verifier rule)
                        dr = work.tile([65, 512], f32r, tag="dsrr", bufs=1,
                                       name="dsrr")
                        nc.vector.tensor_copy(dr[:], dstash[h2][:])
                        dsrr[h2] = dr
                    return go

                def drain_b(xi, h2):
                    def go():
                        b0 = 32 + 32 * xi
                        bc = psp.tile([128, 512], f32, tag="w", bufs=2,
                                      name="bc")
                        nc.tensor.matmul(bc[0:DH, :],
                                         ones_q[b0:b0 + 1, :],
                                         dsrr[h2][b0:b0 + 1, :],
                                         start=True, stop=True)
                        st = work.tile([DH, 512], f32, tag="stg", bufs=2,
                                       name="stg")
                        nc.vector.tensor_copy(st[:], po[(xi, h2)][0:DH, :])
                        bcs[(xi, h2)] = (bc, st)
                    return go

                def drain_c(xi, h2):
                    def go():
                        bc, st = bcs[(xi, h2)]
                        nc.vector.tensor_mul(
                            aoT[p][xi * DH:(xi + 1) * DH,
                                   h2 * 512:(h2 + 1) * 512],
                            st[:], bc[0:DH, :])
                    return go

                for h2 in (0, 1):
                    for c in range(NT):
                        for xi in (0, 1):
                            items.append(pv_mm(xi, h2, c))
                    items.append(drain_a(0, h2))
                    items.append(drain_a(1, h2))
                    items.append(drain_r(h2))
                    items.append(drain_b(0, h2))
                    items.append(drain_b(1, h2))
                    items.append(drain_c(0, h2))
                    items.append(drain_c(1, h2))
                return items

            def pair_step(p, qk, bg=()):
                """S(p) chunk-interleaved with background items; returns
                PT(p). bg items are spread over the 16 (c, xi) half-slots."""
                qt, kt = qk["q"], qk["k"]
                L = len(bg)
                PT = {}
                for c in range(NT):
                    for xi in range(2):
                        ro = xi * 64
                        ps = psp.tile([128, SEQ], f32, tag="s", bufs=2,
                                      name="ps_s")
                        for h2 in range(2):
                            nc.tensor.matmul(
                                ps[:, h2 * 512:(h2 + 1) * 512],
                                kt[ro:ro + 64, c * 128:(c + 1) * 128],
                                qt[ro:ro + 64, h2 * 512:(h2 + 1) * 512],
                                start=True, stop=True)
                        pt = persist.tile([128, SEQ], bf16,
                                          tag=f"pt{p % 2}_{xi}_{c}",
                                          name=f"pt{p}_{xi}_{c}")
                        nc.scalar.activation(pt[:], ps[:], EXP)
                        PT[(xi, c)] = pt
                        hs = 2 * c + xi
                        for i in range(L * hs // 16, L * (hs + 1) // 16):
                            bg[i]()
                return PT

            def merge(a, b):
                out, ia, ib = [], 0, 0
                while ia < len(a) or ib < len(b):
                    if ia * len(b) <= ib * len(a) and ia < len(a):
                        out.append(a[ia]); ia += 1
                    elif ib < len(b):
                        out.append(b[ib]); ib += 1
                    else:
                        out.append(a[ia]); ia += 1
                return out

            # ---- schedule ----
            qk_tiles = {p: {} for p in range(6)}
            run_parts(wt_qkv_parts(0, qk_tiles[0]))

            v_h0 = [v_part(nt, 0) for nt in range(NT)]
            v_h1 = [v_part(nt, 1) for nt in range(NT)]

            bg_sched = {
                0: merge(wt_qkv_parts(1, qk_tiles[1]), v_h0),
                1: merge(wt_qkv_parts(2, qk_tiles[2]), v_h1[0:2]),
                2: merge(wt_qkv_parts(3, qk_tiles[3]), v_h1[2:4]),
                3: merge(wt_qkv_parts(4, qk_tiles[4]), v_h1[4:6]),
                4: merge(wt_qkv_parts(5, qk_tiles[5]), v_h1[6:8]),
                5: wfc_parts(),
            }
            PT_prev = None
            for p in range(6):
                bg = list(bg_sched[p])
                if PT_prev is not None:
                    bg = merge(pv_work(p - 1, PT_prev), bg)
                PT_prev = pair_step(p, qk_tiles[p], bg)
            # tail: PV(5) + drains back-to-back
            run_parts(pv_work(5, PT_prev))

            # ---- fc + bias, natural layout ----
            for nt in range(NT):
                psy = psp.tile([128, SEQ], f32, tag="s", bufs=2, name="psy")
                for j in range(DT):
                    nc.tensor.matmul(psy[:, 0:512],
                                     aoT[j][:, nt * 128:(nt + 1) * 128],
                                     wfcT[j][:, 0:512],
                                     start=(j == 0), stop=(j == DT - 1))
                    nc.tensor.matmul(psy[:, 512:768],
                                     aoT[j][:, nt * 128:(nt + 1) * 128],
                                     wfcT[j][:, 512:768],
                                     start=(j == 0), stop=(j == DT - 1))
                y = work.tile([128, DIM], f32, tag="y_sb", bufs=2, name="y")
                nc.vector.tensor_add(y[:], psy[:, 0:DIM], bias_bc[:])
                nc.sync.dma_start(out_d.ap()[nt * 128:(nt + 1) * 128, :], y[:])

    nc.compile()
    return nc


_NC = None
LAST_RESULTS = None  # BassKernelResults of the most recent run (for profiling)


def kernel(**inputs) -> np.ndarray:
    global _NC, LAST_RESULTS
    x = np.ascontiguousarray(np.asarray(inputs["x"], dtype=np.float32))
    w_qkv = np.ascontiguousarray(np.asarray(inputs["w_qkv"], dtype=np.float32))
    w_fc = np.ascontiguousarray(np.asarray(inputs["w_fc"], dtype=np.float32))
    b_fc = np.ascontiguousarray(
        np.asarray(inputs["b_fc"], dtype=np.float32).reshape(1, DIM))
    eye = np.eye(128, dtype=np.float32)

    if _NC is None:
        _NC = build()
    nc = _NC

    in_maps = [
        {"x": np.ascontiguousarray(x[b]), "w_qkv": w_qkv, "w_fc": w_fc,
         "b_fc": b_fc, "eye": eye}
        for b in range(8)
    ]
    res = run_bass_kernel_spmd(nc, in_maps, core_ids=list(range(8)))
    LAST_RESULTS = res
    out = np.stack([r["out"] for r in res.results], axis=0)
    return out.astype(np.float32)


if __name__ == "__main__":
    rng = np.random.default_rng(0)
    ins = {
        "x": rng.standard_normal((8, SEQ, DIM), dtype=np.float32),
        "w_qkv": (rng.standard_normal((E, DIM), dtype=np.float32) * DIM ** -0.5),
        "w_fc": (rng.standard_normal((DIM, DIM), dtype=np.float32) * DIM ** -0.5),
        "b_fc": (rng.standard_normal((DIM,), dtype=np.float32) * 0.02),
    }
    out = kernel(**ins)
    print("out", out.shape, out.dtype)


# revision 20
# speedup vs baseline: 1.4805x; 1.2182x over previous
"""Trainium2 Bass kernel: fused multi-head attention (dense transformer block).

Reference computation (per batch element b of 8, one NeuronCore each):
    qkv = x @ w_qkv.T                  # [1024, 2304]
    q, k, v = split(qkv); reshape to 12 heads x 64 dims
    s = q @ k.T (unscaled); p = softmax(s); o = p @ v
    out = concat_heads(o) @ w_fc.T + b_fc

Layout strategy (per core): transposed layout throughout — q_T/k_T are
[head_dim, seq], scores S_T[k, q] (keys on partitions), softmax denominator
via ones-column in V. Softmax skips max-subtraction (|scores| < 88).

v2 restructure (vs. 279us baseline): the kernel is paced by two engines in
dead heat — PE matmuls (~21us/head-pair at full clock) and ScalarE's exp
stream (~20.5us/pair). The baseline coupled every background matmul to the
exp drain through one shared 2-buffer PSUM tag, so PE stalled ~1.3us behind
exp constantly, dropping the PE DVFS p-state (2.4 -> 1.2 GHz) and looping.
Fixes:
  - PSUM split into 3 independent groups: scores "s" 2x[128,1024] (4 banks)
    <-> exp ping-pong; PV "pv" 2x[128,512] (2 banks); background "w"
    2x[128,512] (2 banks) drained by VectorE only.
  - P tiles (exp output) double-buffered by pair parity so exp never waits
    on the PV consumer.
  - PV split into q-half groups (h2) with mid-pair drains so it fits 2 banks.
  - Normalization without DRAM bounce and without ScalarE: DVE reciprocal
    reads the denominator row straight from PSUM, a 1-partition PE matmul
    (ones[1,64] x recip[1,512]) broadcasts it, DVE multiplies. ScalarE runs
    exp ONLY (its floor ~123us stays under the PE's ~165us).
  - Background work (qkv for pair p+1, v, w_fc prep, bias) rebalanced across
    pairs so every pair has PE work >= exp work, keeping the PE dense and
    the clock at max.
"""

import numpy as np
import concourse.bacc as bacc
import concourse.mybir as mybir
import concourse.tile as tile
from concourse.bass_utils import run_bass_kernel_spmd

SEQ = 1024
DIM = 768
H = 12
DH = 64
E = 3 * DIM  # 2304
NT = SEQ // 128  # 8  seq chunks
DT = DIM // 128  # 6  dim chunks
VA = H * (DH + 1)  # 780: v with ones column per head

f32 = mybir.dt.float32
f32r = mybir.dt.float32r
bf16 = mybir.dt.bfloat16
EXP = mybir.ActivationFunctionType.Exp


def build():
    nc = bacc.Bacc("TRN2", target_bir_lowering=False, debug=False)
    x_d = nc.dram_tensor("x", [SEQ, DIM], f32r, kind="ExternalInput")
    wqkv_d = nc.dram_tensor("w_qkv", [E, DIM], f32r,
                            kind="ExternalInput")
    wfc_d = nc.dram_tensor("w_fc", [DIM, DIM], f32r,
                           kind="ExternalInput")
    bfc_d = nc.dram_tensor("b_fc", [1, DIM], f32, kind="ExternalInput")
    eye_d = nc.dram_tensor("eye", [128, 128], f32r, kind="ExternalInput")
    out_d = nc.dram_tensor("out", [SEQ, DIM], f32, kind="ExternalOutput")

    with tile.TileContext(nc) as tc:
        with (
            tc.tile_pool(name="const", bufs=1) as constp,
            tc.tile_pool(name="persist", bufs=1) as persist,
            tc.tile_pool(name="work", bufs=1) as work,
            tc.tile_pool(name="ps", bufs=1, space="PSUM") as psp,
        ):
            # ---- constants ----
            eye = constp.tile([128, 128], f32r, tag="eye")
            nc.sync.dma_start(eye[:], eye_d.ap())
            ones_f = constp.tile([1, 128], f32, tag="onesf")
            nc.gpsimd.memset(ones_f[:], 1.0)
            ones_r = constp.tile([1, 128], f32r, tag="onesr")
            nc.vector.tensor_copy(ones_r[:], ones_f[:])
            ones_qf = constp.tile([65, DH], f32, tag="onesqf")
            nc.gpsimd.memset(ones_qf[:], 1.0)
            ones_q = constp.tile([65, DH], f32r, tag="onesq")
            nc.vector.tensor_copy(ones_q[:], ones_qf[:])
            # den stash per q-half: xi0's den lands at partition 32 (33-row
            # copy from base 0), xi1's at 64 (single-row copy). ONE wide
            # reciprocal [65,512] covers both (1-partition DVE recip
            # measured 3.3us; this shape ~0.6us). Rows 33..63 are seeded
            # once below so the wide recip never reads uninitialized
            # memory. SBUF engine APs must start at partition 0/32/64 and
            # respect 32/64-row group spans; PSUM sources are exempt.
            dstash = [constp.tile([65, 512], f32, tag=f"dst{h}",
                                  name=f"dstash{h}")
                      for h in range(2)]
            bias_bc = constp.tile([128, DIM], f32, tag="bbc")

            def bias_prep():
                bias_row = constp.tile([1, DIM], f32, tag="brow")
                nc.sync.dma_start(bias_row[:], bfc_d.ap())
                bias_r = constp.tile([1, DIM], f32r, tag="biasr")
                nc.vector.tensor_copy(bias_r[:], bias_row[:])
                for q in range(DT):
                    bb = psp.tile([128, 512], f32, tag="w", bufs=2, name="bb")
                    nc.tensor.matmul(bb[:, 0:128], ones_r[:],
                                     bias_r[:, q * 128:(q + 1) * 128],
                                     start=True, stop=True)
                    nc.vector.tensor_copy(bias_bc[:, q * 128:(q + 1) * 128],
                                          bb[:, 0:128])

            # persistent tensors
            va = [persist.tile([128, VA], bf16, tag=f"va{nt}", name=f"va{nt}")
                  for nt in range(NT)]
            aoT = [persist.tile([128, SEQ], bf16, tag=f"ao{j}", name=f"aoT{j}")
                   for j in range(DT)]
            wfcT = [persist.tile([128, DIM], bf16, tag=f"wfcT{j}",
                                 name=f"wfcT{j}") for j in range(DT)]
            xT = [persist.tile([128, SEQ], f32r, tag=f"xT{j}", name=f"xT{j}")
                  for j in range(DT)]
            wvT = [persist.tile([128, DIM], f32r, tag=f"wvT{j}",
                                name=f"wvT{j}") for j in range(DT)]

            # ---- x and w_v: load + transpose, groups interleaved so each
            # group's DMA loads hide under the previous group's transposes ----
            def x_group(g):
                xr4 = []
                for i in range(4):
                    nt = g * 4 + i
                    xr = work.tile([128, DIM], f32r, tag=f"u{i}", bufs=1,
                                   name=f"xr{nt}")
                    nc.sync.dma_start(xr[:],
                                      x_d.ap()[nt * 128:(nt + 1) * 128, :])
                    xr4.append(xr)

                def tr():
                    for j in range(DT):
                        tag = ("w", "pv")[j % 2]
                        tp = psp.tile([128, 512], f32r, tag=tag, bufs=2,
                                      name="tp")
                        for i in range(4):
                            nc.tensor.transpose(
                                tp[:, i * 128:(i + 1) * 128],
                                xr4[i][:, j * 128:(j + 1) * 128], eye[:])
                        nc.vector.tensor_copy(
                            xT[j][:, g * 512:(g + 1) * 512], tp[:])
                return tr

            def wv_group(g):
                # pairs of w_v rows staged in the fc-tail's y_sb buffers
                # (disjoint lifetimes) so these DMAs run parallel to the x
                # loads instead of queueing behind the x staging tags
                wr2 = []
                for i in range(2 * g, 2 * g + 2):
                    wr = work.tile([128, DIM], f32r, tag="y_sb", bufs=2,
                                   name=f"wvr{i}")
                    nc.sync.dma_start(
                        wr[:], wqkv_d.ap()[(12 + i) * 128:(13 + i) * 128, :])
                    wr2.append(wr)

                def tr():
                    for j in range(DT):
                        tag = ("w", "pv")[j % 2]
                        tp = psp.tile([128, 512], f32r, tag=tag, bufs=2,
                                      name="tp")
                        for ii in range(2):
                            nc.tensor.transpose(
                                tp[:, ii * 128:(ii + 1) * 128],
                                wr2[ii][:, j * 128:(j + 1) * 128], eye[:])
                        nc.vector.tensor_copy(
                            wvT[j][:, 2 * g * 128:(2 * g + 2) * 128],
                            tp[:, 0:256])
                return tr

            def seed_dstash():
                for h in range(2):
                    nc.vector.tensor_copy(dstash[h][0:DH, :],
                                          xT[0][0:DH, 0:512])

            tr_x0 = x_group(0)
            tr_wvA = wv_group(0)
            tr_x0()
            tr_x1 = x_group(1)
            tr_wvA()
            tr_wvB = wv_group(1)
            tr_x1()
            tr_wvC = wv_group(2)
            tr_wvB()
            tr_wvC()
            seed_dstash()

            # ---- v matmuls as self-contained parts (one (nt, h2) each) ----
            def v_part(nt, h2):
                lo, hi = (0, 512) if h2 == 0 else (512, 768)

                def go():
                    psv = psp.tile([128, 512], f32, tag="w", bufs=2,
                                   name="psv")
                    for j in range(DT):
                        nc.tensor.matmul(psv[:, 0:hi - lo],
                                         xT[j][:, nt * 128:(nt + 1) * 128],
                                         wvT[j][:, lo:hi],
                                         start=(j == 0), stop=(j == DT - 1))
                    va3 = va[nt][:].rearrange("p (h c) -> p h c", c=DH + 1)
                    if h2 == 0:
                        # ones columns for ALL heads now: PV(p) reads head
                        # 2p's ones col as early as pair 1, but h2==1 v parts
                        # land as late as pair 4.
                        nc.gpsimd.memset(va3[:, :, DH:DH + 1], 1.0)
                    nc.vector.tensor_copy(
                        va3[:, lo // DH:hi // DH, 0:DH],
                        psv[:, 0:hi - lo].rearrange("p (h c) -> p h c", c=DH))
                return go

            def wfc_parts():
                """w_fc load + PE-transpose as self-contained parts."""
                frs = {}

                def load(idxs):
                    for ii, ft in enumerate(idxs):
                        fr = work.tile([128, DIM], f32r, tag=f"u{2 + ii}",
                                       bufs=1, name=f"fr{ft}")
                        nc.sync.dma_start(
                            fr[:], wfc_d.ap()[ft * 128:(ft + 1) * 128, :])
                        frs[ii] = fr

                def tgrp(idxs, js):
                    for j in js:
                        tp = psp.tile([128, 512], f32r, tag="w", bufs=2,
                                      name="tp")
                        for ii in range(len(idxs)):
                            nc.tensor.transpose(
                                tp[:, ii * 128:(ii + 1) * 128],
                                frs[ii][:, j * 128:(j + 1) * 128], eye[:])
                        nc.vector.tensor_copy(
                            wfcT[j][:, idxs[0] * 128:
                                   (idxs[0] + len(idxs)) * 128],
                            tp[:, 0:128 * len(idxs)])

                g0, g1 = range(0, 4), range(4, 6)
                return [bias_prep,
                        lambda: (load(g0), tgrp(g0, range(0, 3))),
                        lambda: tgrp(g0, range(3, 6)),
                        lambda: (load(g1), tgrp(g1, range(0, 3))),
                        lambda: tgrp(g1, range(3, 6))]

            def wt_qkv_parts(p, tiles):
                """Pair p's w_qkv load/transpose + q_T/k_T matmuls as
                self-contained parts; results appear in `tiles`."""
                wq_t = [work.tile([128, 256], f32r, tag=f"wq{j}", bufs=1,
                                  name=f"wq{j}_{p}") for j in range(DT)]
                wraws = {}

                def load():
                    for ci, et in enumerate((p, 6 + p)):
                        wraw = work.tile([128, DIM], f32r, tag=f"u{ci}",
                                         bufs=1, name=f"wqr{et}")
                        nc.sync.dma_start(
                            wraw[:], wqkv_d.ap()[et * 128:(et + 1) * 128, :])
                        wraws[ci] = wraw

                def tgrp(js):
                    for j in js:
                        tp = psp.tile([128, 512], f32r, tag="w", bufs=2,
                                      name="tp")
                        for ci in range(2):
                            nc.tensor.transpose(
                                tp[:, ci * 128:(ci + 1) * 128],
                                wraws[ci][:, j * 128:(j + 1) * 128],
                                eye[:])
                        nc.vector.tensor_copy(wq_t[j][:], tp[:, 0:256])

                def qkmm(ci, half, h2):
                    ps = psp.tile([128, 512], f32, tag="w", bufs=2,
                                  name="psq")
                    for j in range(DT):
                        nc.tensor.matmul(
                            ps[:],
                            wq_t[j][:, ci * 128:(ci + 1) * 128],
                            xT[j][:, h2 * 512:(h2 + 1) * 512],
                            start=(j == 0), stop=(j == DT - 1))
                    if h2 == 0:
                        t = work.tile([128, SEQ], f32r,
                                      tag=f"qk_{half}{p % 2}", bufs=1,
                                      name=f"qk{half}{p}")
                        tiles[half] = t
                    nc.vector.tensor_copy(
                        tiles[half][:, h2 * 512:(h2 + 1) * 512], ps[:])

                return [lambda: (load(), tgrp(range(0, 3))),
                        lambda: tgrp(range(3, 6)),
                        lambda: qkmm(0, "q", 0), lambda: qkmm(0, "q", 1),
                        lambda: qkmm(1, "k", 0), lambda: qkmm(1, "k", 1)]

            def run_parts(parts):
                for f in parts:
                    f()

            # ---- PV + normalize machinery ----
            def pv_work(p, PT):
                """Closures for PV(p): per (h2): 16 c-x-xi matmuls into
                [65,512] po tiles (ones-row gives the softmax denominator),
                then drains. Drain = DVE recip (PSUM in) -> 1-partition PE
                broadcast matmul -> DVE multiply into aoT. No ScalarE, no
                DRAM bounce."""
                po = {}
                bcs = {}
                dsrr = {}
                items = []

                def pv_mm(xi, h2, c):
                    def go():
                        if c == 0:
                            po[(xi, h2)] = psp.tile(
                                [128, 512], f32, tag="pv", bufs=2,
                                name=f"po{xi}_{h2}")
                        hX = 2 * p + xi
                        va_h = va[c][:, hX * (DH + 1):(hX + 1) * (DH + 1)]
                        nc.tensor.matmul(
                            po[(xi, h2)][0:DH + 1, :], va_h,
                            PT[(xi, c)][:, h2 * 512:(h2 + 1) * 512],
                            start=(c == 0), stop=(c == NT - 1))
                    return go

                def drain_a(xi, h2):
                    def go():
                        b0 = 32 + 32 * xi
                        nc.vector.tensor_copy(dstash[h2][b0:b0 + 1, :],
                                              po[(xi, h2)][DH:DH + 1, :])
                    return go

                def drain_r(h2):
                    def go():
                        # DVE reciprocal costs ~6.4ns/free-element regardless
                        # of partitions (measured 3.3us for 512-wide);
                        # approx_fast is a single custom-DVE op (~18 good
                        # bits, den is in [1,1e33] so edge cases are safe)
                        nc.vector.reciprocal_approx_fast(
                            out=dstash[h2][:], in_=dstash[h2][:])
                        # rounded f32r copy: the f32r broadcast matmul needs
                        # an f32r-producing instruction (BIR verifier rule)
                        dr = work.tile([65, 512], f32r, tag="dsrr", bufs=1,
                                       name="dsrr")
                        nc.vector.tensor_copy(dr[:], dstash[h2][:])
                        dsrr[h2] = dr
                    return go

                def drain_b(xi, h2):
                    def go():
                        b0 = 32 + 32 * xi
                        bc = psp.tile([128, 512], f32, tag="w", bufs=2,
                                      name="bc")
                        nc.tensor.matmul(bc[0:DH, :],
                                         ones_q[b0:b0 + 1, :],
                                         dsrr[h2][b0:b0 + 1, :],
                                         start=True, stop=True)
                        st = work.tile([DH, 512], f32, tag="stg", bufs=2,
                                       name="stg")
                        nc.vector.tensor_copy(st[:], po[(xi, h2)][0:DH, :])
                        bcs[(xi, h2)] = (bc, st)
                    return go

                def drain_c(xi, h2):
                    def go():
                        bc, st = bcs[(xi, h2)]
                        nc.vector.tensor_mul(
                            aoT[p][xi * DH:(xi + 1) * DH,
                                   h2 * 512:(h2 + 1) * 512],
                            st[:], bc[0:DH, :])
                    return go

                for h2 in (0, 1):
                    for c in range(NT):
                        for xi in (0, 1):
                            items.append(pv_mm(xi, h2, c))
                    items.append(drain_a(0, h2))
                    items.append(drain_a(1, h2))
                    items.append(drain_r(h2))
                    items.append(drain_b(0, h2))
                    items.append(drain_b(1, h2))
                    items.append(drain_c(0, h2))
                    items.append(drain_c(1, h2))
                return items

            def pair_step(p, qk, bg=()):
                """S(p) chunk-interleaved with background items; returns
                PT(p). bg items are spread over the 16 (c, xi) half-slots."""
                qt, kt = qk["q"], qk["k"]
                L = len(bg)
                PT = {}
                for c in range(NT):
                    for xi in range(2):
                        ro = xi * 64
                        ps = psp.tile([128, SEQ], f32, tag="s", bufs=2,
                                      name="ps_s")
                        for h2 in range(2):
                            nc.tensor.matmul(
                                ps[:, h2 * 512:(h2 + 1) * 512],
                                kt[ro:ro + 64, c * 128:(c + 1) * 128],
                                qt[ro:ro + 64, h2 * 512:(h2 + 1) * 512],
                                start=True, stop=True)
                        pt = persist.tile([128, SEQ], bf16,
                                          tag=f"pt{p % 2}_{xi}_{c}",
                                          name=f"pt{p}_{xi}_{c}")
                        nc.scalar.activation(pt[:], ps[:], EXP)
                        PT[(xi, c)] = pt
                        hs = 2 * c + xi
                        for i in range(L * hs // 16, L * (hs + 1) // 16):
                            bg[i]()
                return PT

            def merge(a, b):
                out, ia, ib = [], 0, 0
                while ia < len(a) or ib < len(b):
                    if ia * len(b) <= ib * len(a) and ia < len(a):
                        out.append(a[ia]); ia += 1
                    elif ib < len(b):
                        out.append(b[ib]); ib += 1
                    else:
                        out.append(a[ia]); ia += 1
                return out

            # ---- schedule ----
            qk_tiles = {p: {} for p in range(6)}
            run_parts(wt_qkv_parts(0, qk_tiles[0]))

            v_h0 = [v_part(nt, 0) for nt in range(NT)]
            v_h1 = [v_part(nt, 1) for nt in range(NT)]

            bg_sched = {
                0: merge(wt_qkv_parts(1, qk_tiles[1]), v_h0),
                1: merge(wt_qkv_parts(2, qk_tiles[2]), v_h1[0:2]),
                2: merge(wt_qkv_parts(3, qk_tiles[3]), v_h1[2:4]),
                3: merge(wt_qkv_parts(4, qk_tiles[4]), v_h1[4:6]),
                4: merge(wt_qkv_parts(5, qk_tiles[5]), v_h1[6:8]),
                5: wfc_parts(),
            }
            PT_prev = None
            for p in range(6):
                bg = list(bg_sched[p])
                if PT_prev is not None:
                    bg = merge(pv_work(p - 1, PT_prev), bg)
                PT_prev = pair_step(p, qk_tiles[p], bg)
            # tail: PV(5) + drains back-to-back
            run_parts(pv_work(5, PT_prev))

            # ---- fc + bias, natural layout ----
            for nt in range(NT):
                psy = psp.tile([128, SEQ], f32, tag="s", bufs=2, name="psy")
                for j in range(DT):
                    nc.tensor.matmul(psy[:, 0:512],
                                     aoT[j][:, nt * 128:(nt + 1) * 128],
                                     wfcT[j][:, 0:512],
                                     start=(j == 0), stop=(j == DT - 1))
                    nc.tensor.matmul(psy[:, 512:768],
                                     aoT[j][:, nt * 128:(nt + 1) * 128],
                                     wfcT[j][:, 512:768],
                                     start=(j == 0), stop=(j == DT - 1))
                y = work.tile([128, DIM], f32, tag="y_sb", bufs=2, name="y")
                nc.vector.tensor_add(y[:], psy[:, 0:DIM], bias_bc[:])
                nc.sync.dma_start(out_d.ap()[nt * 128:(nt + 1) * 128, :], y[:])

    nc.compile()
    return nc


_NC = None
LAST_RESULTS = None  # BassKernelResults of the most recent run (for profiling)


def kernel(**inputs) -> np.ndarray:
    global _NC, LAST_RESULTS
    x = np.ascontiguousarray(np.asarray(inputs["x"], dtype=np.float32))
    w_qkv = np.ascontiguousarray(np.asarray(inputs["w_qkv"], dtype=np.float32))
    w_fc = np.ascontiguousarray(np.asarray(inputs["w_fc"], dtype=np.float32))
    b_fc = np.ascontiguousarray(
        np.asarray(inputs["b_fc"], dtype=np.float32).reshape(1, DIM))
    eye = np.eye(128, dtype=np.float32)

    if _NC is None:
        _NC = build()
    nc = _NC

    in_maps = [
        {"x": np.ascontiguousarray(x[b]), "w_qkv": w_qkv, "w_fc": w_fc,
         "b_fc": b_fc, "eye": eye}
        for b in range(8)
    ]
    res = run_bass_kernel_spmd(nc, in_maps, core_ids=list(range(8)))
    LAST_RESULTS = res
    out = np.stack([r["out"] for r in res.results], axis=0)
    return out.astype(np.float32)


if __name__ == "__main__":
    rng = np.random.default_rng(0)
    ins = {
        "x": rng.standard_normal((8, SEQ, DIM), dtype=np.float32),
        "w_qkv": (rng.standard_normal((E, DIM), dtype=np.float32) * DIM ** -0.5),
        "w_fc": (rng.standard_normal((DIM, DIM), dtype=np.float32) * DIM ** -0.5),
        "b_fc": (rng.standard_normal((DIM,), dtype=np.float32) * 0.02),
    }
    out = kernel(**ins)
    print("out", out.shape, out.dtype)
